# revision 21
# baseline (speedup 1.0000x reference)
"""CoAttention Trainium2 kernel (bf16 I/O, fused epilogue).

Computes A[b,i,j] = u[b,i,:]@w1 + v[b,j,:]@w2 + sum_d u[b,i,d]*w3[d]*v[b,j,d]
for u, v: [16, 2048, 256] f32, w1/w2/w3: [256] f32 -> A: [16, 2048, 2048] f32.

Sharding: batch dim (16) split across 8 NeuronCores (2 batches/core, data
parallel); w1/w2/w3 replicated.

Memory-regime strategy: the kernel is HBM-bound (output is 256 MiB), so all
device I/O is bf16 (rel-err gate is 2e-2; bf16 end-to-end lands ~3e-3):
  - host pre-transposes u,v to [D, S] layout and casts to bf16 (removes all
    PE transposes and halves input DMA)
  - output tensor is bf16 (halves the dominant store traffic), host upcasts
Device per batch:
  - vw3T[d,j] = w3[d]*vT[d,j] on DVE (per-partition scale, bf16 fast path)
  - w2vb[p,j] = sum_d w2[d] vT[d,j] via PE (w2 replicated stationary)
  - w1u[i] = sum_d uT[d,i] w1[d] via tiny N=1 matmuls (uT chunk stationary)
  - per 128-row i-block: psum[i,j] += uT_chunk^T @ vw3T_chunk (bf16 matmuls
    stream at ~216ns/512-row on the PE)
PSUM is organized as [128,1024] half-tiles with a 4-deep rotation so the
epilogue drain latency hides behind 3 half-tiles of PE work. Drain paths
(GPSIMD cannot read PSUM; ACT can only add per-partition bias):
  a) DVE scalar_tensor_tensor: orow = (psum + w1u[i]) + w2vb   (~1.3us)
  b) ACT bias (psum+w1u -> orow bf16), GpSimd orow += w2vb     (ACT 1.1 + GP 2.1)
  c) ACT bias (psum+w1u -> orow bf16), DVE orow += w2vb        (ACT 1.1 + DVE ~0.4)
mixed a:c:b = 2:2:1 to balance DVE/ACT/GP busy under the PE roofline.
One 512 KiB bf16 store per i-block on the sync ring.
"""

import numpy as np
from contextlib import ExitStack

B, S, D = 16, 2048, 256
N_CORES = 8
BPC = B // N_CORES  # batches per core
P = 128
NB = S // P    # 16 i blocks
NCH = D // P   # 2 contraction chunks
FQ = 512       # matmul psum slice (one bank)
HQ = 1024      # psum half-tile width
NH = S // HQ   # 2 halves per i-block

_CACHE = {}


def _build(level=40):
    import concourse.bacc as bacc
    import concourse.mybir as mybir
    import concourse.tile as tile

    dt = mybir.dt
    f32 = dt.float32
    bf16 = dt.bfloat16
    ADD = mybir.AluOpType.add
    MULT = mybir.AluOpType.mult
    IDENT = mybir.ActivationFunctionType.Identity
    COPY = mybir.ActivationFunctionType.Copy

    nc = bacc.Bacc("TRN2", debug=False, num_devices=N_CORES)
    ut_d = nc.dram_tensor("ut", [BPC, D, S], bf16, kind="ExternalInput").ap()
    vt_d = nc.dram_tensor("vt", [BPC, D, S], bf16, kind="ExternalInput").ap()
    w1t_d = nc.dram_tensor("w1t", [P, NCH], bf16, kind="ExternalInput").ap()
    # w2/w3 in column layout, merged into one load (descriptor-gen is
    # ~650ns per DMA regardless of size — fewer tiny loads ahead of the
    # input transfers)
    wsc_d = nc.dram_tensor("wsc", [P, 2, NCH], f32, kind="ExternalInput").ap()
    out_d = nc.dram_tensor("out", [BPC, S, S], bf16, kind="ExternalOutput").ap()

    with tile.TileContext(nc) as tc, ExitStack() as ctx:
        const = ctx.enter_context(tc.tile_pool(name="const", bufs=1))
        inp = ctx.enter_context(tc.tile_pool(name="inp", bufs=2))
        vw_pool = ctx.enter_context(tc.tile_pool(name="vw", bufs=2))
        work = ctx.enter_context(tc.tile_pool(name="work", bufs=2))
        outp = ctx.enter_context(tc.tile_pool(name="outp", bufs=4))
        psp = ctx.enter_context(tc.tile_pool(name="psp", bufs=4, space="PSUM"))

        # ---- constants first: tiny loads, must not queue behind the
        # 4.2 MB input transfers. All loads ride the sync ring; the scalar
        # ring's ACT_TABLE_LOAD would delay dispatch by ~1.3us.
        wsc = const.tile([P, 2, NCH], f32, tag="wsc")
        nc.sync.dma_start(out=wsc[:], in_=wsc_d)
        w2tc = wsc[:, 0, :]
        w3t = wsc[:, 1, :]
        w1t = const.tile([P, NCH], bf16, tag="w1t")
        nc.sync.dma_start(out=w1t[:], in_=w1t_d)
        ones = const.tile([P, P], bf16, tag="ones")
        nc.vector.memset(ones[:], 1.0)
        warm = const.tile([P, FQ], bf16, tag="warm")
        nc.vector.memset(warm[:], 0.0)

        # PE p-state warmup: the PE only reaches full clock after ~3us of
        # continuous work, and any idle gap drops it back. Burn dummy
        # matmuls through the otherwise-idle input-DMA window so the real
        # stream starts (and stays) at full clock.
        ps_warm = psp.tile([P, FQ], f32, tag="ps", name="ps_warm")
        for _ in range(32):
            nc.tensor.matmul(
                ps_warm[:], lhsT=warm[:, :P], rhs=warm[:],
                start=True, stop=True,
            )

        # ut0 before vt0: ut0+w1t gate the PE w1u phase, which fills the
        # window while vt0 is still in flight (vw3/w2vb need vt0).
        vt_sbs, ut_sbs = [], []
        for bi in range(BPC):
            vt_sb = inp.tile([P, NCH, S], bf16, tag="vt", name=f"vt{bi}")
            ut_sb = inp.tile([P, NCH, S], bf16, tag="ut", name=f"ut{bi}")
            if bi == 0:
                nc.sync.dma_start(
                    out=ut_sb[:],
                    in_=ut_d[bi].rearrange("(ch p) s -> p ch s", p=P),
                )
                nc.sync.dma_start(
                    out=vt_sb[:],
                    in_=vt_d[bi].rearrange("(ch p) s -> p ch s", p=P),
                )
            else:
                nc.sync.dma_start(
                    out=vt_sb[:],
                    in_=vt_d[bi].rearrange("(ch p) s -> p ch s", p=P),
                )
                nc.sync.dma_start(
                    out=ut_sb[:],
                    in_=ut_d[bi].rearrange("(ch p) s -> p ch s", p=P),
                )
            vt_sbs.append(vt_sb)
            ut_sbs.append(ut_sb)

        # w2t[d, ch, p] = w2[ch*128+d] (stationary operand for the w2v
        # broadcast: psum[p, j] += sum_d w2t[d,p] * vT[d,j])
        w2t = const.tile([P, NCH, P], bf16, tag="w2t")
        for ch in range(NCH):
            nc.vector.tensor_scalar(
                w2t[:, ch, :], ones[:], w2tc[:, ch:ch + 1], None, MULT,
            )

        def build_batch(bi):
            """vw3 (DVE), w1u + w2vb (PE + drains) for batch bi."""
            vt_sb, ut_sb = vt_sbs[bi], ut_sbs[bi]

            # w1u[i] = sum_d uT[d,i] w1[d]; one N=1 matmul per (ib, ch).
            # These pipeline at ~27ns spacing on the PE (~1us total).
            ps_w1 = psp.tile([P, NB], f32, tag="ps", name=f"ps_w1u_{bi}")
            for ib in range(NB):
                for ch in range(NCH):
                    nc.tensor.matmul(
                        ps_w1[:, ib:ib + 1],
                        lhsT=ut_sb[:, ch, ib * P:(ib + 1) * P],
                        rhs=w1t[:, ch:ch + 1],
                        start=(ch == 0), stop=(ch == NCH - 1),
                    )
            w1u = work.tile([P, NB], f32, tag="w1u", name=f"w1u_{bi}")
            nc.vector.tensor_copy(w1u[:], ps_w1[:])

            if bi == 0:
                # bridge the vt0-arrival gap so the warmed-up PE clock
                # does not drop before the main stream begins
                ps_wm = psp.tile([P, FQ], f32, tag="ps", name="ps_warm2")
                for _ in range(9):
                    nc.tensor.matmul(
                        ps_wm[:], lhsT=warm[:, :P], rhs=warm[:],
                        start=True, stop=True,
                    )

            # vw3T[d, j] = w3[d] * vT[d, j]  (DVE per-partition scale)
            vw3 = vw_pool.tile([P, NCH, S], bf16, tag="vw3", name=f"vw3_{bi}")
            for ch in range(NCH):
                nc.vector.tensor_scalar(
                    vw3[:, ch, :], vt_sb[:, ch, :], w3t[:, ch:ch + 1], None,
                    MULT,
                )

            # w2vb[p, j] = w2v[j] for all p (PE broadcast matmul, 2 halves)
            w2vb = work.tile([P, S], bf16, tag="w2vb", name=f"w2vb_{bi}")
            for jh in range(NH):
                ps_w = psp.tile([P, HQ], f32, tag="ps", name=f"ps_w2v_{bi}_{jh}")
                for q in range(2):
                    qs_p = slice(q * FQ, (q + 1) * FQ)
                    qs_v = slice(jh * HQ + q * FQ, jh * HQ + (q + 1) * FQ)
                    for ch in range(NCH):
                        nc.tensor.matmul(
                            ps_w[:, qs_p], lhsT=w2t[:, ch, :],
                            rhs=vt_sb[:, ch, qs_v],
                            start=(ch == 0), stop=(ch == NCH - 1),
                        )
                nc.scalar.activation(
                    out=w2vb[:, jh * HQ:(jh + 1) * HQ], in_=ps_w[:], func=COPY
                )
            return vw3, w2vb, w1u

        built = {0: build_batch(0)}
        for bi in range(BPC):
            vw3, w2vb, w1u = built[bi]
            ut_sb = ut_sbs[bi]
            for ib in range(NB):
                # emit the next batch's builds a couple of i-blocks before
                # this batch ends: the PE slots them into its in-order
                # stream with no idle (no p-state reset at the boundary)
                if ib == NB - 2 and bi + 1 < BPC:
                    built[bi + 1] = build_batch(bi + 1)
                orow = outp.tile([P, S], bf16, tag="orow")
                for jh in range(NH):
                    ps = psp.tile(
                        [P, HQ], f32, tag="ps", name=f"ps_{bi}_{ib}_{jh}"
                    )
                    # ch-outer: stationary uT chunk held across both slices
                    for ch in range(NCH):
                        for q in range(2):
                            qs_p = slice(q * FQ, (q + 1) * FQ)
                            qs_v = slice(
                                jh * HQ + q * FQ, jh * HQ + (q + 1) * FQ
                            )
                            nc.tensor.matmul(
                                ps[:, qs_p],
                                lhsT=ut_sb[:, ch, ib * P:(ib + 1) * P],
                                rhs=vw3[:, ch, qs_v],
                                start=(ch == 0), stop=(ch == NCH - 1),
                            )
                    js = slice(jh * HQ, (jh + 1) * HQ)
                    idx = ib * NH + jh
                    # a:c:b = 2:2:1. GpSimd traffic contends with DVE/PE
                    # on the shared SBUF ports (more 'b' slows every
                    # engine ~17%), so keep its share small. Final halves
                    # forced to 'a' (shortest drain chain) to cut the tail.
                    if bi == BPC - 1 and idx >= 2 * NB - 2:
                        path = "a"
                    else:
                        path = ("a", "c", "a", "c", "b")[idx % 5]
                    if path == "a":
                        nc.vector.scalar_tensor_tensor(
                            out=orow[:, js], in0=ps[:],
                            scalar=w1u[:, ib:ib + 1],
                            in1=w2vb[:, js], op0=ADD, op1=ADD,
                        )
                    else:
                        nc.scalar.activation(
                            out=orow[:, js], in_=ps[:], func=IDENT,
                            bias=w1u[:, ib:ib + 1], scale=1.0,
                        )
                        eng = nc.vector if path == "c" else nc.gpsimd
                        eng.tensor_tensor(
                            out=orow[:, js], in0=orow[:, js],
                            in1=w2vb[:, js], op=ADD,
                        )
                nc.sync.dma_start(
                    out=out_d[bi, ib * P:(ib + 1) * P, :], in_=orow[:]
                )

    nc.compile()
    return nc


def _get_nc():
    if "nc" not in _CACHE:
        _CACHE["nc"] = _build()
    return _CACHE["nc"]


def kernel(u, v, w1, w2, w3, _trace=False, _trace_cores=None, _results_out=None):
    import ml_dtypes
    from concourse.bass_utils import run_bass_kernel_spmd

    bf16 = ml_dtypes.bfloat16
    nc = _get_nc()

    # host-side layout prep: cast to bf16, transpose to [D, S]
    ut = np.ascontiguousarray(
        np.asarray(u, dtype=np.float32).astype(bf16).transpose(0, 2, 1)
    )
    vt = np.ascontiguousarray(
        np.asarray(v, dtype=np.float32).astype(bf16).transpose(0, 2, 1)
    )
    w1t = np.ascontiguousarray(
        np.asarray(w1, dtype=np.float32).reshape(NCH, P).T
    ).astype(bf16)
    # wsc[:, 0, :] = w2 col layout, wsc[:, 1, :] = w3 col layout
    wsc = np.stack(
        [
            np.asarray(w2, dtype=np.float32).reshape(NCH, P).T,
            np.asarray(w3, dtype=np.float32).reshape(NCH, P).T,
        ],
        axis=1,
    ).astype(np.float32)
    wsc = np.ascontiguousarray(wsc)

    in_maps = [
        {
            "ut": np.ascontiguousarray(ut[c * BPC:(c + 1) * BPC]),
            "vt": np.ascontiguousarray(vt[c * BPC:(c + 1) * BPC]),
            "w1t": w1t,
            "wsc": wsc,
        }
        for c in range(N_CORES)
    ]
    kw = {}
    if _trace:
        kw["trace"] = True
        if _trace_cores is not None:
            kw["trace_cores"] = _trace_cores
    res = run_bass_kernel_spmd(nc, in_maps, core_ids=list(range(N_CORES)), **kw)
    if _results_out is not None:
        _results_out.append(res)
    out = np.concatenate(
        [np.asarray(res.results[c]["out"]) for c in range(N_CORES)], axis=0
    )
    return out.astype(np.float32)
